# revision 1
# baseline (speedup 1.0000x reference)
"""Trainium2 Bass kernel for nn_CausalityConstraints.

Strategy (pure data parallel, B=8 batch elements -> 8 NeuronCores):

The only heavy input is aspect_opinion_relations [8,1024,1024,4] f32 (128 MB).
The reference needs just two booleans per (b, i):
    full_any[b,i]    = any(rel[b,i,:,:]  > 0.5)   (== maxrel > 0.5)
    earlier_any[b,i] = any(rel[b,i,:i,:] > 0.5)   (== exists_earlier)
Each core reduces its 16 MB slice at the DMA roofline. Rows i sit on the 128
partitions (8 row-tiles of [128, 4096] = [p, (opinion_pos, k)]):
  * ScalarE computes the full-row "any" as sum(relu(x - 0.5)) > 0 with the
    fused accumulate output (exact: x - 0.5 is Sterbenz-exact for
    x in [0.25, 1), negative otherwise, so relu > 0 iff x > 0.5), in 4
    quarter-row pieces pipelined behind 4 quarter-row DMAs per tile.
  * VectorE computes the "earlier" masked max: a plain reduce over the
    fully-earlier prefix columns plus a strictly-lower-triangular-masked
    max over the 512-wide diagonal chunk (mask multiply + reduce;
    tensor_tensor_reduce / tensor_mask_reduce crash on this runtime).
Row-tiles are processed t = 7..0 so the final tile's epilogue is minimal.

Everything else is O(B*S) work on [8,1024] vectors (MLP factors, window
tests, and the 1024-step sequential scan).  The scan has a closed form: with
per-position "updated-row activity" candidates u0/u1 (isolated vs not), the
recurrence  a_i = !k_i & (src_i | a_{i-1} | a_{i-2})  is reachability that is
blocked only by two consecutive "kill" positions, so
    a_i = u1_i & (last_src_pos_i >= last_double_kill_pos_i, src exists)
with both "last positions" plain prefix maxima.  This is evaluated
vectorized on the host (microseconds); the device does the 128 MB part.
"""

import numpy as np

B = 8
S = 1024
NT = 8            # row tiles of 128 rows each
CW = 512          # diagonal chunk width (= 128 opinion positions * K)
NSPLIT = 4        # DMA/ScalarE pieces per row-tile
W = S * 4
OUTW = NSPLIT * NT + NT

_CACHE = {}


# --------------------------------------------------------------------------
# device kernel
# --------------------------------------------------------------------------

def _build_nc(repeat=0, internal_rel=False):
    """repeat=0: plain kernel (production).  repeat>=1: wrap the whole pass
    in a hardware For_i loop (for marginal-time measurement); internal_rel
    puts rel in internal scratch DRAM so invocations skip the 16MB upload."""
    import contextlib
    import concourse.bacc as bacc
    import concourse.tile as tile
    import concourse.mybir as mybir

    nc = bacc.Bacc("TRN2", target_bir_lowering=False, debug=False, num_devices=B)
    f32 = mybir.dt.float32
    AX = mybir.AxisListType
    OP = mybir.AluOpType
    ACT = mybir.ActivationFunctionType
    if internal_rel:
        rel = nc.dram_tensor("relscratch", [S, W], f32)
    else:
        rel = nc.dram_tensor("rel", [S, W], f32, kind="ExternalInput")
    trimask = nc.dram_tensor("trimask", [128, CW], f32, kind="ExternalInput")
    outmax = nc.dram_tensor("outmax", [128, OUTW], f32, kind="ExternalOutput")
    PW = W // NSPLIT

    with tile.TileContext(nc) as tc:
        with tc.tile_pool(name="relp", bufs=4) as relp, \
             tc.tile_pool(name="small", bufs=1) as small, \
             tc.tile_pool(name="scr", bufs=2) as scr, \
             tc.tile_pool(name="parts", bufs=4) as parts:
            mask = small.tile([128, CW], f32)
            nc.sync.dma_start(out=mask[:, :], in_=trimask[:, :])
            biasm = small.tile([128, 1], f32)
            nc.vector.memset(biasm[:, :], -0.5)
            dummy = small.tile([128, 1], f32)
            out_t = small.tile([128, OUTW], f32)
            nc.scalar.memzero(out_t[:, :])
            loop_ctx = tc.For_i(0, repeat, 1) if repeat else contextlib.nullcontext()
            with loop_ctx:
                for t in reversed(range(NT)):
                    rt = relp.tile([128, W], f32, tag="rt")
                    rows = rel[t * 128:(t + 1) * 128, :]
                    for q in range(NSPLIT):
                        nc.sync.dma_start(out=rt[:, q * PW:(q + 1) * PW],
                                          in_=rows[:, q * PW:(q + 1) * PW])
                        # ScalarE "any" only over the suffix [512t, 4096):
                        # columns below 512t are covered by VectorE's prefix
                        # max via  full_any = earlier_any | suffix_any.
                        lo = max(q * PW, t * CW)
                        hi = (q + 1) * PW
                        if lo >= hi:
                            continue
                        nc.scalar.activation(
                            out=dummy.broadcast_to((128, hi - lo)),
                            in_=rt[:, lo:hi],
                            func=ACT.Relu, bias=biasm[:, :], scale=1.0,
                            accum_out=out_t[:, NSPLIT * t + q:NSPLIT * t + q + 1])
                    sc = scr.tile([128, CW], f32)
                    diag = parts.tile([128, 1], f32)
                    nc.vector.tensor_tensor(
                        out=sc[:, :], in0=rt[:, t * CW:(t + 1) * CW],
                        in1=mask[:, :], op=OP.mult)
                    oc = NSPLIT * NT + t
                    if t > 0:
                        nc.vector.tensor_reduce(
                            out=diag[:, :], in_=sc[:, :], axis=AX.X, op=OP.max)
                        pref = parts.tile([128, 1], f32)
                        nc.vector.tensor_reduce(
                            out=pref[:, :], in_=rt[:, 0:t * CW], axis=AX.X, op=OP.max)
                        nc.vector.tensor_tensor(
                            out=out_t[:, oc:oc + 1], in0=pref[:, :],
                            in1=diag[:, :], op=OP.max)
                    else:
                        nc.vector.tensor_reduce(
                            out=out_t[:, oc:oc + 1], in_=sc[:, :], axis=AX.X, op=OP.max)
            nc.sync.dma_start(out=outmax[:, :], in_=out_t[:, :])
    nc.compile()
    return nc


def _get_nc():
    if "nc" not in _CACHE:
        _CACHE["nc"] = _build_nc()
    return _CACHE["nc"]


def _trimask():
    if "mask" not in _CACHE:
        li = np.arange(128)[:, None]          # local row index
        f = np.arange(CW)[None, :]            # free index = local_p*4 + k
        _CACHE["mask"] = (f < 4 * li).astype(np.float32)
    return _CACHE["mask"]


def unpack_outmax(om):
    """om: [128, OUTW] -> (full_any [S], earlier_any [S]) bools; row i = t*128+p.

    ScalarE sums only cover the suffix columns [512t, 4096); the prefix is
    covered by the VectorE earlier-max, and the masked diagonal lies inside
    the suffix, so  full_any = earlier_any | suffix_any  exactly."""
    sums = om[:, :NSPLIT * NT].reshape(128, NT, NSPLIT)      # [p, t, q]
    suffix = (sums > 0.0).any(-1).T.reshape(S)               # [t, p] -> i
    earlier = om[:, NSPLIT * NT:].T.reshape(S) > 0.5
    return suffix | earlier, earlier


def run_device(rel, trace=False):
    """rel: [B, S, S, 4] f32.  Returns (full_any, earlier_any [B,S] bool, results)."""
    from concourse.bass_utils import run_bass_kernel_spmd

    nc = _get_nc()
    mask = _trimask()
    in_maps = [
        {"rel": np.ascontiguousarray(rel[b].reshape(S, W)), "trimask": mask}
        for b in range(B)
    ]
    res = run_bass_kernel_spmd(nc, in_maps, core_ids=list(range(B)), trace=trace)
    full = np.empty((B, S), dtype=bool)
    earlier = np.empty((B, S), dtype=bool)
    for b, r in enumerate(res.results):
        full[b], earlier[b] = unpack_outmax(r["outmax"])
    return full, earlier, res


# --------------------------------------------------------------------------
# host epilogue: O(B*S) factor logic + closed-form scan
# --------------------------------------------------------------------------

def _host_forward(aL, oL, full_any, earlier_any, exA, exO,
                  W1, b1, W2, b2, W3, b3):
    B_, S_, _ = aL.shape
    x = np.concatenate([aL, oL], axis=-1)
    h = np.maximum(x @ W1 + b1, 0.0).astype(np.float32)
    h = np.maximum(h @ W2 + b2, 0.0).astype(np.float32)
    z = (h @ W3 + b3)[..., 0].astype(np.float32)
    c = (1.0 / (1.0 + np.exp(-z.astype(np.float64)))).astype(np.float32)
    mult1 = np.where(c < 0.5, np.float32(2.0) * c, np.float32(1.0)).astype(np.float32)

    def window_any(flag, w):
        out = np.zeros_like(flag)
        for d in range(-w, w + 1):
            if d < 0:
                out[:, :d] |= flag[:, -d:]
            elif d > 0:
                out[:, d:] |= flag[:, :-d]
            else:
                out |= flag
        return out

    nearA = window_any(exA > 0, 3)
    nearO = window_any(exO > 0, 3)
    e = np.exp((aL - aL.max(-1, keepdims=True)).astype(np.float32))
    impA = e[..., :2].sum(-1) / e.sum(-1)
    e = np.exp((oL - oL.max(-1, keepdims=True)).astype(np.float32))
    impO = e[..., :2].sum(-1) / e.sum(-1)
    factA2 = np.where((impA > 0.5) & ~nearO, np.float32(0.3), np.float32(1.0))
    factO2 = np.where((impO > 0.5) & ~nearA, np.float32(0.3), np.float32(1.0))

    factA4 = np.where(full_any & earlier_any, np.float32(0.7), np.float32(1.0))

    # ---- scan closed form ----
    actO = (aL.max(-1) > 0.5) | (oL.max(-1) > 0.5)
    n = np.zeros((B_, S_), dtype=bool)
    n[:, :-1] |= actO[:, 1:]
    n[:, :-2] |= actO[:, 2:]

    def act_of(fa, fo):
        aRow = aL * mult1[..., None]
        oRow = oL * mult1[..., None]
        aRow[..., :2] *= fa[..., None]
        oRow[..., :2] *= fo[..., None]
        return (aRow.max(-1) > 0.5) | (oRow.max(-1) > 0.5)

    u1 = act_of((factA2 * np.float32(1.0)) * factA4, factO2 * np.float32(1.0))
    u0 = act_of((factA2 * np.float32(0.1)) * factA4, factO2 * np.float32(0.1))

    k = ~u1
    src = u0 | (u1 & n)
    DK = np.zeros((B_, S_), dtype=bool)
    DK[:, 1:] = k[:, 1:] & k[:, :-1]

    idx = np.arange(S_)[None, :]
    LS = np.maximum.accumulate(np.where(src, idx, -1), axis=1)
    LDK = np.maximum.accumulate(np.where(DK, idx, -1), axis=1)
    a = u1 & (LS >= 0) & (LS >= LDK)

    r = n.copy()
    r[:, 1:] |= a[:, :-1]
    r[:, 2:] |= a[:, :-2]
    fact3 = np.where(~r, np.float32(0.1), np.float32(1.0))

    fa = (factA2 * fact3) * factA4
    fo = factO2 * fact3
    cA = aL * mult1[..., None]
    cO = oL * mult1[..., None]
    cA[..., :2] *= fa[..., None]
    cO[..., :2] *= fo[..., None]
    return cA.astype(np.float32), cO.astype(np.float32)


# --------------------------------------------------------------------------
# entry point
# --------------------------------------------------------------------------

def kernel(aspect_logits, opinion_logits, aspect_opinion_relations,
           explicit_aspects, explicit_opinions, W1, b1, W2, b2, W3, b3):
    aL = np.asarray(aspect_logits, dtype=np.float32)
    oL = np.asarray(opinion_logits, dtype=np.float32)
    rel = np.asarray(aspect_opinion_relations, dtype=np.float32)
    exA = np.asarray(explicit_aspects)
    exO = np.asarray(explicit_opinions)
    full_any, earlier_any, _ = run_device(rel)
    return _host_forward(
        aL, oL, full_any, earlier_any, exA, exO,
        np.asarray(W1, np.float32), np.asarray(b1, np.float32),
        np.asarray(W2, np.float32), np.asarray(b2, np.float32),
        np.asarray(W3, np.float32), np.asarray(b3, np.float32))



# revision 6
# speedup vs baseline: 1.0248x; 1.0248x over previous
"""Trainium2 Bass kernel for nn_CausalityConstraints.

Strategy (pure data parallel, B=8 batch elements -> 8 NeuronCores):

The only heavy input is aspect_opinion_relations [8,1024,1024,4] f32 (128 MB).
The reference needs just two booleans per (b, i):
    full_any[b,i]    = any(rel[b,i,:,:]  > 0.5)   (== maxrel > 0.5)
    earlier_any[b,i] = any(rel[b,i,:i,:] > 0.5)   (== exists_earlier)
Each core reduces its 16 MB slice at the HBM-per-core roofline (358 GB/s):
the slice is viewed as [1024 rows, 4096] and moved as 8 row-tile DMAs of
[128, 4096] (2 MB contiguous each, all on the SP HWDGE ring: measured
355 GB/s sustained vs 307 GB/s for 4x512KB pieces).  Compute is split three
ways so no engine ever gates the DMA stream (each ~11-17 us vs 47 us DMA):
  * ScalarE: suffix "any" over cols [512t, 4096) as sum(relu(x - 0.5)) > 0
    with the fused accumulate output (exact: x - 0.5 is Sterbenz-exact for
    x in [0.25, 1), negative otherwise, so relu > 0 iff x > 0.5), plus the
    same accumulate trick for the prefix columns of tiles t in {6, 7}.
  * VectorE: plain max-reduce over the fully-earlier prefix columns
    [0, 512t) for t in {1..5}, and the strictly-lower-triangular-masked
    max over the 512-wide diagonal chunk of every tile (mask multiply +
    reduce; tensor_tensor_reduce / tensor_mask_reduce crash on this
    runtime, and GpSimd's tensor_reduce is partition-axis only).
full_any = earlier_any | suffix_any.  Row-tiles are ordered so the final
tile is t=2, whose per-engine epilogue is balanced (~2.7 us).

Everything else is O(B*S) work on [8,1024] vectors (MLP factors, window
tests, and the 1024-step sequential scan).  The scan has a closed form: with
per-position "updated-row activity" candidates u0/u1 (isolated vs not), the
recurrence  a_i = !k_i & (src_i | a_{i-1} | a_{i-2})  is reachability that is
blocked only by two consecutive "kill" positions, so
    a_i = u1_i & (last_src_pos_i >= last_double_kill_pos_i, src exists)
with both "last positions" plain prefix maxima.  This is evaluated
vectorized on the host (microseconds); the device does the 128 MB part.
"""

import numpy as np

B = 8
S = 1024
NT = 8            # row tiles of 128 rows each
CW = 512          # diagonal chunk width (= 128 opinion positions * K)
W = S * 4
OUTW = 3 * NT     # per tile: suffix-sum, prefix-max-or-sum, diag-max
TILE_ORDER = [7, 6, 5, 4, 3, 1, 0, 2]
SCALAR_PREFIX = (6, 7)   # prefix "any" via ScalarE accum; rest via DVE max

_CACHE = {}


# --------------------------------------------------------------------------
# device kernel
# --------------------------------------------------------------------------

def _build_nc(repeat=0, internal_rel=False):
    """repeat=0: plain kernel (production).  repeat>=1: wrap the whole pass
    in a hardware For_i loop (for marginal-time measurement); internal_rel
    puts rel in internal scratch DRAM so invocations skip the 16MB upload."""
    import contextlib
    import concourse.bacc as bacc
    import concourse.tile as tile
    import concourse.mybir as mybir

    nc = bacc.Bacc("TRN2", target_bir_lowering=False, debug=False, num_devices=B)
    f32 = mybir.dt.float32
    AX = mybir.AxisListType
    OP = mybir.AluOpType
    ACT = mybir.ActivationFunctionType
    if internal_rel:
        rel = nc.dram_tensor("relscratch", [S, W], f32)
    else:
        rel = nc.dram_tensor("rel", [S, W], f32, kind="ExternalInput")
    trimask = nc.dram_tensor("trimask", [128, CW], f32, kind="ExternalInput")
    outmax = nc.dram_tensor("outmax", [128, OUTW], f32, kind="ExternalOutput")

    with tile.TileContext(nc) as tc:
        with tc.tile_pool(name="relp", bufs=NT) as relp, \
             tc.tile_pool(name="small", bufs=1) as small, \
             tc.tile_pool(name="scr", bufs=2) as scr:
            mask = small.tile([128, CW], f32)
            nc.sync.dma_start(out=mask[:, :], in_=trimask[:, :])
            biasm = small.tile([128, 1], f32)
            nc.vector.memset(biasm[:, :], -0.5)
            dummy = small.tile([128, 1], f32)
            out_t = small.tile([128, OUTW], f32)
            nc.scalar.memzero(out_t[:, :])
            loop_ctx = tc.For_i(0, repeat, 1) if repeat else contextlib.nullcontext()
            with loop_ctx:
                for t in TILE_ORDER:
                    rt = relp.tile([128, W], f32, tag="rt")
                    nc.sync.dma_start(out=rt[:, :],
                                      in_=rel[t * 128:(t + 1) * 128, :])
                    # ScalarE: "any" over the suffix [512t, 4096); columns
                    # below 512t are covered by the prefix max via
                    # full_any = earlier_any | suffix_any.
                    nc.scalar.activation(
                        out=dummy.broadcast_to((128, W - t * CW)),
                        in_=rt[:, t * CW:],
                        func=ACT.Relu, bias=biasm[:, :], scale=1.0,
                        accum_out=out_t[:, t:t + 1])
                    # Prefix "any" over the fully-earlier columns [0, 512t).
                    if t in SCALAR_PREFIX:
                        nc.scalar.activation(
                            out=dummy.broadcast_to((128, t * CW)),
                            in_=rt[:, 0:t * CW],
                            func=ACT.Relu, bias=biasm[:, :], scale=1.0,
                            accum_out=out_t[:, NT + t:NT + t + 1])
                    elif t > 0:
                        nc.vector.tensor_reduce(
                            out=out_t[:, NT + t:NT + t + 1],
                            in_=rt[:, 0:t * CW], axis=AX.X, op=OP.max)
                    # VectorE: masked max over the diagonal chunk.
                    sc = scr.tile([128, CW], f32)
                    nc.vector.tensor_tensor(
                        out=sc[:, :], in0=rt[:, t * CW:(t + 1) * CW],
                        in1=mask[:, :], op=OP.mult)
                    nc.vector.tensor_reduce(
                        out=out_t[:, 2 * NT + t:2 * NT + t + 1],
                        in_=sc[:, :], axis=AX.X, op=OP.max)
            nc.sync.dma_start(out=outmax[:, :], in_=out_t[:, :])
    nc.compile()
    return nc


def _get_nc():
    if "nc" not in _CACHE:
        _CACHE["nc"] = _build_nc()
    return _CACHE["nc"]


def _trimask():
    if "mask" not in _CACHE:
        li = np.arange(128)[:, None]          # local row index
        f = np.arange(CW)[None, :]            # free index = local_p*4 + k
        _CACHE["mask"] = (f < 4 * li).astype(np.float32)
    return _CACHE["mask"]


def unpack_outmax(om):
    """om: [128, 3*NT] -> (full_any [S], earlier_any [S]) bools; row i = t*128+p.

    ScalarE sums only cover the suffix columns [512t, 4096); the prefix is
    covered by the GpSimd earlier-max, and the masked diagonal lies inside
    the suffix, so  full_any = earlier_any | suffix_any  exactly."""
    suffix = (om[:, :NT] > 0.0).T.reshape(S)                 # [t, p] -> i
    pthr = np.where(np.isin(np.arange(NT), SCALAR_PREFIX), 0.0, 0.5)
    earlier = ((om[:, NT:2 * NT] > pthr[None, :]) |
               (om[:, 2 * NT:] > 0.5)).T.reshape(S)
    return suffix | earlier, earlier


def run_device(rel, trace=False):
    """rel: [B, S, S, 4] f32.  Returns (full_any, earlier_any [B,S] bool, results)."""
    from concourse.bass_utils import run_bass_kernel_spmd

    nc = _get_nc()
    mask = _trimask()
    in_maps = [
        {"rel": np.ascontiguousarray(rel[b].reshape(S, W)), "trimask": mask}
        for b in range(B)
    ]
    res = run_bass_kernel_spmd(nc, in_maps, core_ids=list(range(B)), trace=trace)
    full = np.empty((B, S), dtype=bool)
    earlier = np.empty((B, S), dtype=bool)
    for b, r in enumerate(res.results):
        full[b], earlier[b] = unpack_outmax(r["outmax"])
    return full, earlier, res


# --------------------------------------------------------------------------
# host epilogue: O(B*S) factor logic + closed-form scan
# --------------------------------------------------------------------------

def _host_forward(aL, oL, full_any, earlier_any, exA, exO,
                  W1, b1, W2, b2, W3, b3):
    B_, S_, _ = aL.shape
    x = np.concatenate([aL, oL], axis=-1)
    h = np.maximum(x @ W1 + b1, 0.0).astype(np.float32)
    h = np.maximum(h @ W2 + b2, 0.0).astype(np.float32)
    z = (h @ W3 + b3)[..., 0].astype(np.float32)
    c = (1.0 / (1.0 + np.exp(-z.astype(np.float64)))).astype(np.float32)
    mult1 = np.where(c < 0.5, np.float32(2.0) * c, np.float32(1.0)).astype(np.float32)

    def window_any(flag, w):
        out = np.zeros_like(flag)
        for d in range(-w, w + 1):
            if d < 0:
                out[:, :d] |= flag[:, -d:]
            elif d > 0:
                out[:, d:] |= flag[:, :-d]
            else:
                out |= flag
        return out

    nearA = window_any(exA > 0, 3)
    nearO = window_any(exO > 0, 3)
    e = np.exp((aL - aL.max(-1, keepdims=True)).astype(np.float32))
    impA = e[..., :2].sum(-1) / e.sum(-1)
    e = np.exp((oL - oL.max(-1, keepdims=True)).astype(np.float32))
    impO = e[..., :2].sum(-1) / e.sum(-1)
    factA2 = np.where((impA > 0.5) & ~nearO, np.float32(0.3), np.float32(1.0))
    factO2 = np.where((impO > 0.5) & ~nearA, np.float32(0.3), np.float32(1.0))

    factA4 = np.where(full_any & earlier_any, np.float32(0.7), np.float32(1.0))

    # ---- scan closed form ----
    actO = (aL.max(-1) > 0.5) | (oL.max(-1) > 0.5)
    n = np.zeros((B_, S_), dtype=bool)
    n[:, :-1] |= actO[:, 1:]
    n[:, :-2] |= actO[:, 2:]

    def act_of(fa, fo):
        aRow = aL * mult1[..., None]
        oRow = oL * mult1[..., None]
        aRow[..., :2] *= fa[..., None]
        oRow[..., :2] *= fo[..., None]
        return (aRow.max(-1) > 0.5) | (oRow.max(-1) > 0.5)

    u1 = act_of((factA2 * np.float32(1.0)) * factA4, factO2 * np.float32(1.0))
    u0 = act_of((factA2 * np.float32(0.1)) * factA4, factO2 * np.float32(0.1))

    k = ~u1
    src = u0 | (u1 & n)
    DK = np.zeros((B_, S_), dtype=bool)
    DK[:, 1:] = k[:, 1:] & k[:, :-1]

    idx = np.arange(S_)[None, :]
    LS = np.maximum.accumulate(np.where(src, idx, -1), axis=1)
    LDK = np.maximum.accumulate(np.where(DK, idx, -1), axis=1)
    a = u1 & (LS >= 0) & (LS >= LDK)

    r = n.copy()
    r[:, 1:] |= a[:, :-1]
    r[:, 2:] |= a[:, :-2]
    fact3 = np.where(~r, np.float32(0.1), np.float32(1.0))

    fa = (factA2 * fact3) * factA4
    fo = factO2 * fact3
    cA = aL * mult1[..., None]
    cO = oL * mult1[..., None]
    cA[..., :2] *= fa[..., None]
    cO[..., :2] *= fo[..., None]
    return cA.astype(np.float32), cO.astype(np.float32)


# --------------------------------------------------------------------------
# entry point
# --------------------------------------------------------------------------

def kernel(aspect_logits, opinion_logits, aspect_opinion_relations,
           explicit_aspects, explicit_opinions, W1, b1, W2, b2, W3, b3):
    aL = np.asarray(aspect_logits, dtype=np.float32)
    oL = np.asarray(opinion_logits, dtype=np.float32)
    rel = np.asarray(aspect_opinion_relations, dtype=np.float32)
    exA = np.asarray(explicit_aspects)
    exO = np.asarray(explicit_opinions)
    full_any, earlier_any, _ = run_device(rel)
    return _host_forward(
        aL, oL, full_any, earlier_any, exA, exO,
        np.asarray(W1, np.float32), np.asarray(b1, np.float32),
        np.asarray(W2, np.float32), np.asarray(b2, np.float32),
        np.asarray(W3, np.float32), np.asarray(b3, np.float32))


# revision 10
# speedup vs baseline: 1.0411x; 1.0159x over previous
"""Trainium2 Bass kernel for nn_CausalityConstraints.

Strategy (pure data parallel, B=8 batch elements -> 8 NeuronCores):

The only heavy input is aspect_opinion_relations [8,1024,1024,4] f32 (128 MB).
The reference needs just two booleans per (b, i):
    full_any[b,i]    = any(rel[b,i,:,:]  > 0.5)   (== maxrel > 0.5)
    earlier_any[b,i] = any(rel[b,i,:i,:] > 0.5)   (== exists_earlier)
Each core reduces its 16 MB slice at the HBM-per-core roofline (358 GB/s):
the slice is viewed as [1024 rows, 4096] and moved as 8 row-tile DMAs of
[128, 4096] (2 MB contiguous each, all on the SP HWDGE ring: measured
355 GB/s sustained vs 307 GB/s for 4x512KB pieces).  Compute is split three
ways so no engine ever gates the DMA stream (each ~11-17 us vs 47 us DMA):
  * ScalarE: suffix "any" over cols [512t, 4096) as sum(relu(x - 0.5)) > 0
    with the fused accumulate output (exact: x - 0.5 is Sterbenz-exact for
    x in [0.25, 1), negative otherwise, so relu > 0 iff x > 0.5), plus the
    same accumulate trick for the prefix columns of tiles t in {6, 7}.
  * VectorE: plain max-reduce over the fully-earlier prefix columns
    [0, 512t) for t in {1..5}, and the strictly-lower-triangular-masked
    max over the 512-wide diagonal chunk of every tile (mask multiply +
    reduce; tensor_tensor_reduce / tensor_mask_reduce crash on this
    runtime, and GpSimd's tensor_reduce is partition-axis only).
full_any = earlier_any | suffix_any.  Row-tiles are ordered so the final
tile is t=2, and its 2 MB DMA is split [0,2816) + [2816,4096) so most of
its epilogue overlaps the last 0.6 MB of streaming (~1.2 us exposed).

Everything else is O(B*S) work on [8,1024] vectors (MLP factors, window
tests, and the 1024-step sequential scan).  The scan has a closed form: with
per-position "updated-row activity" candidates u0/u1 (isolated vs not), the
recurrence  a_i = !k_i & (src_i | a_{i-1} | a_{i-2})  is reachability that is
blocked only by two consecutive "kill" positions, so
    a_i = u1_i & (last_src_pos_i >= last_double_kill_pos_i, src exists)
with both "last positions" plain prefix maxima.  This is evaluated
vectorized on the host (microseconds); the device does the 128 MB part.
"""

import numpy as np

B = 8
S = 1024
NT = 8            # row tiles of 128 rows each
CW = 512          # diagonal chunk width (= 128 opinion positions * K)
W = S * 4
OUTW = 3 * NT + 1  # per tile: suffix-sum, prefix-max-or-sum, diag-max;
                   # +1: second suffix piece of the split last tile
TILE_ORDER = [7, 6, 5, 4, 3, 1, 0, 2]
SCALAR_PREFIX = (6, 7)   # prefix "any" via ScalarE accum; rest via DVE max
LAST_SPLIT = 2816        # last tile's DMA split column (piece A = [0, 2816))

_CACHE = {}


# --------------------------------------------------------------------------
# device kernel
# --------------------------------------------------------------------------

def _build_nc(repeat=0, internal_rel=False):
    """repeat=0: plain kernel (production).  repeat>=1: wrap the whole pass
    in a hardware For_i loop (for marginal-time measurement); internal_rel
    puts rel in internal scratch DRAM so invocations skip the 16MB upload."""
    import contextlib
    import concourse.bacc as bacc
    import concourse.tile as tile
    import concourse.mybir as mybir

    nc = bacc.Bacc("TRN2", target_bir_lowering=False, debug=False, num_devices=B)
    f32 = mybir.dt.float32
    AX = mybir.AxisListType
    OP = mybir.AluOpType
    ACT = mybir.ActivationFunctionType
    if internal_rel:
        rel = nc.dram_tensor("relscratch", [S, W], f32)
    else:
        rel = nc.dram_tensor("rel", [S, W], f32, kind="ExternalInput")
    trimask = nc.dram_tensor("trimask", [128, CW], f32, kind="ExternalInput")
    outmax = nc.dram_tensor("outmax", [128, OUTW], f32, kind="ExternalOutput")

    with tile.TileContext(nc) as tc:
        with tc.tile_pool(name="relp", bufs=NT) as relp, \
             tc.tile_pool(name="small", bufs=1) as small, \
             tc.tile_pool(name="scr", bufs=2) as scr:
            mask = small.tile([128, CW], f32)
            nc.sync.dma_start(out=mask[:, :], in_=trimask[:, :])
            biasm = small.tile([128, 1], f32)
            nc.vector.memset(biasm[:, :], -0.5)
            dummy = small.tile([128, 1], f32)
            out_t = small.tile([128, OUTW], f32)
            nc.scalar.memzero(out_t[:, :])
            loop_ctx = tc.For_i(0, repeat, 1) if repeat else contextlib.nullcontext()
            with loop_ctx:
                for t in TILE_ORDER:
                    last = t == TILE_ORDER[-1]
                    rt = relp.tile([128, W], f32, tag="rt")
                    if last:
                        nc.sync.dma_start(out=rt[:, :LAST_SPLIT],
                                          in_=rel[t * 128:(t + 1) * 128, :LAST_SPLIT])
                    else:
                        nc.sync.dma_start(out=rt[:, :],
                                          in_=rel[t * 128:(t + 1) * 128, :])
                    # ScalarE: "any" over the suffix [512t, 4096); columns
                    # below 512t are covered by the prefix max via
                    # full_any = earlier_any | suffix_any.
                    if last:
                        nc.scalar.activation(
                            out=dummy.broadcast_to((128, LAST_SPLIT - t * CW)),
                            in_=rt[:, t * CW:LAST_SPLIT],
                            func=ACT.Relu, bias=biasm[:, :], scale=1.0,
                            accum_out=out_t[:, t:t + 1])
                    else:
                        nc.scalar.activation(
                            out=dummy.broadcast_to((128, W - t * CW)),
                            in_=rt[:, t * CW:],
                            func=ACT.Relu, bias=biasm[:, :], scale=1.0,
                            accum_out=out_t[:, t:t + 1])
                    # Prefix "any" over the fully-earlier columns [0, 512t).
                    if t in SCALAR_PREFIX:
                        nc.scalar.activation(
                            out=dummy.broadcast_to((128, t * CW)),
                            in_=rt[:, 0:t * CW],
                            func=ACT.Relu, bias=biasm[:, :], scale=1.0,
                            accum_out=out_t[:, NT + t:NT + t + 1])
                    elif t > 0:
                        nc.vector.tensor_reduce(
                            out=out_t[:, NT + t:NT + t + 1],
                            in_=rt[:, 0:t * CW], axis=AX.X, op=OP.max)
                    # VectorE: masked max over the diagonal chunk.
                    sc = scr.tile([128, CW], f32)
                    nc.vector.tensor_tensor(
                        out=sc[:, :], in0=rt[:, t * CW:(t + 1) * CW],
                        in1=mask[:, :], op=OP.mult)
                    nc.vector.tensor_reduce(
                        out=out_t[:, 2 * NT + t:2 * NT + t + 1],
                        in_=sc[:, :], axis=AX.X, op=OP.max)
                    if last:
                        # Piece B of the split last tile: the remaining
                        # suffix columns, streamed + reduced after piece A.
                        nc.sync.dma_start(out=rt[:, LAST_SPLIT:],
                                          in_=rel[t * 128:(t + 1) * 128, LAST_SPLIT:])
                        nc.scalar.activation(
                            out=dummy.broadcast_to((128, W - LAST_SPLIT)),
                            in_=rt[:, LAST_SPLIT:],
                            func=ACT.Relu, bias=biasm[:, :], scale=1.0,
                            accum_out=out_t[:, 3 * NT:3 * NT + 1])
            nc.sync.dma_start(out=outmax[:, :], in_=out_t[:, :])
    nc.compile()
    return nc


def _get_nc():
    if "nc" not in _CACHE:
        _CACHE["nc"] = _build_nc()
    return _CACHE["nc"]


def _trimask():
    if "mask" not in _CACHE:
        li = np.arange(128)[:, None]          # local row index
        f = np.arange(CW)[None, :]            # free index = local_p*4 + k
        _CACHE["mask"] = (f < 4 * li).astype(np.float32)
    return _CACHE["mask"]


def unpack_outmax(om):
    """om: [128, 3*NT] -> (full_any [S], earlier_any [S]) bools; row i = t*128+p.

    ScalarE sums only cover the suffix columns [512t, 4096); the prefix is
    covered by the GpSimd earlier-max, and the masked diagonal lies inside
    the suffix, so  full_any = earlier_any | suffix_any  exactly."""
    sums = om[:, :NT].copy()
    sums[:, TILE_ORDER[-1]] += om[:, 3 * NT]   # second piece of split tile
    suffix = (sums > 0.0).T.reshape(S)                       # [t, p] -> i
    pthr = np.where(np.isin(np.arange(NT), SCALAR_PREFIX), 0.0, 0.5)
    earlier = ((om[:, NT:2 * NT] > pthr[None, :]) |
               (om[:, 2 * NT:2 * NT + NT] > 0.5)).T.reshape(S)
    return suffix | earlier, earlier


def run_device(rel, trace=False):
    """rel: [B, S, S, 4] f32.  Returns (full_any, earlier_any [B,S] bool, results)."""
    from concourse.bass_utils import run_bass_kernel_spmd

    nc = _get_nc()
    mask = _trimask()
    in_maps = [
        {"rel": np.ascontiguousarray(rel[b].reshape(S, W)), "trimask": mask}
        for b in range(B)
    ]
    res = run_bass_kernel_spmd(nc, in_maps, core_ids=list(range(B)), trace=trace)
    full = np.empty((B, S), dtype=bool)
    earlier = np.empty((B, S), dtype=bool)
    for b, r in enumerate(res.results):
        full[b], earlier[b] = unpack_outmax(r["outmax"])
    return full, earlier, res


# --------------------------------------------------------------------------
# host epilogue: O(B*S) factor logic + closed-form scan
# --------------------------------------------------------------------------

def _host_forward(aL, oL, full_any, earlier_any, exA, exO,
                  W1, b1, W2, b2, W3, b3):
    B_, S_, _ = aL.shape
    x = np.concatenate([aL, oL], axis=-1)
    h = np.maximum(x @ W1 + b1, 0.0).astype(np.float32)
    h = np.maximum(h @ W2 + b2, 0.0).astype(np.float32)
    z = (h @ W3 + b3)[..., 0].astype(np.float32)
    c = (1.0 / (1.0 + np.exp(-z.astype(np.float64)))).astype(np.float32)
    mult1 = np.where(c < 0.5, np.float32(2.0) * c, np.float32(1.0)).astype(np.float32)

    def window_any(flag, w):
        out = np.zeros_like(flag)
        for d in range(-w, w + 1):
            if d < 0:
                out[:, :d] |= flag[:, -d:]
            elif d > 0:
                out[:, d:] |= flag[:, :-d]
            else:
                out |= flag
        return out

    nearA = window_any(exA > 0, 3)
    nearO = window_any(exO > 0, 3)
    e = np.exp((aL - aL.max(-1, keepdims=True)).astype(np.float32))
    impA = e[..., :2].sum(-1) / e.sum(-1)
    e = np.exp((oL - oL.max(-1, keepdims=True)).astype(np.float32))
    impO = e[..., :2].sum(-1) / e.sum(-1)
    factA2 = np.where((impA > 0.5) & ~nearO, np.float32(0.3), np.float32(1.0))
    factO2 = np.where((impO > 0.5) & ~nearA, np.float32(0.3), np.float32(1.0))

    factA4 = np.where(full_any & earlier_any, np.float32(0.7), np.float32(1.0))

    # ---- scan closed form ----
    actO = (aL.max(-1) > 0.5) | (oL.max(-1) > 0.5)
    n = np.zeros((B_, S_), dtype=bool)
    n[:, :-1] |= actO[:, 1:]
    n[:, :-2] |= actO[:, 2:]

    def act_of(fa, fo):
        aRow = aL * mult1[..., None]
        oRow = oL * mult1[..., None]
        aRow[..., :2] *= fa[..., None]
        oRow[..., :2] *= fo[..., None]
        return (aRow.max(-1) > 0.5) | (oRow.max(-1) > 0.5)

    u1 = act_of((factA2 * np.float32(1.0)) * factA4, factO2 * np.float32(1.0))
    u0 = act_of((factA2 * np.float32(0.1)) * factA4, factO2 * np.float32(0.1))

    k = ~u1
    src = u0 | (u1 & n)
    DK = np.zeros((B_, S_), dtype=bool)
    DK[:, 1:] = k[:, 1:] & k[:, :-1]

    idx = np.arange(S_)[None, :]
    LS = np.maximum.accumulate(np.where(src, idx, -1), axis=1)
    LDK = np.maximum.accumulate(np.where(DK, idx, -1), axis=1)
    a = u1 & (LS >= 0) & (LS >= LDK)

    r = n.copy()
    r[:, 1:] |= a[:, :-1]
    r[:, 2:] |= a[:, :-2]
    fact3 = np.where(~r, np.float32(0.1), np.float32(1.0))

    fa = (factA2 * fact3) * factA4
    fo = factO2 * fact3
    cA = aL * mult1[..., None]
    cO = oL * mult1[..., None]
    cA[..., :2] *= fa[..., None]
    cO[..., :2] *= fo[..., None]
    return cA.astype(np.float32), cO.astype(np.float32)


# --------------------------------------------------------------------------
# entry point
# --------------------------------------------------------------------------

def kernel(aspect_logits, opinion_logits, aspect_opinion_relations,
           explicit_aspects, explicit_opinions, W1, b1, W2, b2, W3, b3):
    aL = np.asarray(aspect_logits, dtype=np.float32)
    oL = np.asarray(opinion_logits, dtype=np.float32)
    rel = np.asarray(aspect_opinion_relations, dtype=np.float32)
    exA = np.asarray(explicit_aspects)
    exO = np.asarray(explicit_opinions)
    full_any, earlier_any, _ = run_device(rel)
    return _host_forward(
        aL, oL, full_any, earlier_any, exA, exO,
        np.asarray(W1, np.float32), np.asarray(b1, np.float32),
        np.asarray(W2, np.float32), np.asarray(b2, np.float32),
        np.asarray(W3, np.float32), np.asarray(b3, np.float32))


# revision 12
# speedup vs baseline: 1.0684x; 1.0263x over previous
"""Trainium2 Bass kernel for nn_CausalityConstraints.

Strategy (pure data parallel, B=8 batch elements -> 8 NeuronCores):

The only heavy input is aspect_opinion_relations [8,1024,1024,4] f32 (128 MB).
The reference needs just two booleans per (b, i):
    full_any[b,i]    = any(rel[b,i,:,:]  > 0.5)   (== maxrel > 0.5)
    earlier_any[b,i] = any(rel[b,i,:i,:] > 0.5)   (== exists_earlier)
Each core reduces its 16 MB slice at the HBM-per-core roofline (358 GB/s):
the slice is viewed as [1024 rows, 4096] and moved as 8 row-tile DMAs of
[128, 4096] (2 MB contiguous each, all on the SP HWDGE ring: measured
355 GB/s sustained vs 307 GB/s for 4x512KB pieces).  Compute is split three
ways so no engine ever gates the DMA stream (each ~11-17 us vs 47 us DMA):
  * ScalarE: suffix "any" over cols [512t, 4096) as sum(relu(x - 0.5)) > 0
    with the fused accumulate output (exact: x - 0.5 is Sterbenz-exact for
    x in [0.25, 1), negative otherwise, so relu > 0 iff x > 0.5), plus the
    same accumulate trick for the prefix columns of tiles t in {6, 7}.
  * VectorE: plain max-reduce over the fully-earlier prefix columns
    [0, 512t) for t in {1..5}, and the strictly-lower-triangular-masked
    max over the 512-wide diagonal chunk of every tile (mask multiply +
    reduce; tensor_tensor_reduce / tensor_mask_reduce crash on this
    runtime, and GpSimd's tensor_reduce is partition-axis only).
full_any = earlier_any | suffix_any.  Row-tiles are ordered so the final
tile is t=2, and its 2 MB DMA is split [0,2816) + [2816,4096) so most of
its epilogue overlaps the last 0.6 MB of streaming (~1.2 us exposed).

Everything else is O(B*S) work on [8,1024] vectors (MLP factors, window
tests, and the 1024-step sequential scan).  The scan has a closed form: with
per-position "updated-row activity" candidates u0/u1 (isolated vs not), the
recurrence  a_i = !k_i & (src_i | a_{i-1} | a_{i-2})  is reachability that is
blocked only by two consecutive "kill" positions, so
    a_i = u1_i & (last_src_pos_i >= last_double_kill_pos_i, src exists)
with both "last positions" plain prefix maxima.  This is evaluated
vectorized on the host (microseconds); the device does the 128 MB part.
"""

import numpy as np

B = 8
S = 1024
NT = 8            # row tiles of 128 rows each
CW = 512          # diagonal chunk width (= 128 opinion positions * K)
W = S * 4
OUTW = 3 * NT + 1  # per tile: suffix-sum, prefix-max-or-sum, diag-max;
                   # +1: second suffix piece of the split last tile
TILE_ORDER = [7, 6, 5, 4, 3, 1, 0, 2]
SCALAR_PREFIX = (6, 7)   # prefix "any" via ScalarE accum; rest via DVE max
LAST_SPLIT = 2816        # last tile's DMA split column (piece A = [0, 2816))

_CACHE = {}


# --------------------------------------------------------------------------
# device kernel
# --------------------------------------------------------------------------

def _build_nc(repeat=0, internal_rel=False, unroll=1):
    """repeat=0: plain kernel (production).  repeat>=1: wrap `unroll` full
    passes in a hardware For_i loop (for marginal-time measurement);
    internal_rel puts rel in internal scratch DRAM so invocations skip the
    16MB upload."""
    import contextlib
    import concourse.bacc as bacc
    import concourse.tile as tile
    import concourse.mybir as mybir

    nc = bacc.Bacc("TRN2", target_bir_lowering=False, debug=False, num_devices=B)
    f32 = mybir.dt.float32
    AX = mybir.AxisListType
    OP = mybir.AluOpType
    ACT = mybir.ActivationFunctionType
    if internal_rel:
        rel = nc.dram_tensor("relscratch", [S, W], f32)
    else:
        rel = nc.dram_tensor("rel", [S, W], f32, kind="ExternalInput")
    trimask = nc.dram_tensor("trimask", [128, CW], f32, kind="ExternalInput")
    outmax = nc.dram_tensor("outmax", [128, OUTW], f32, kind="ExternalOutput")

    with tile.TileContext(nc) as tc:
        with tc.tile_pool(name="relp", bufs=NT) as relp, \
             tc.tile_pool(name="small", bufs=1) as small, \
             tc.tile_pool(name="scr", bufs=2) as scr:
            mask = small.tile([128, CW], f32)
            nc.sync.dma_start(out=mask[:, :], in_=trimask[:, :])
            biasm = small.tile([128, 1], f32)
            nc.vector.memset(biasm[:, :], -0.5)
            dummy = small.tile([128, 1], f32)
            out_t = small.tile([128, OUTW], f32)
            nc.scalar.memzero(out_t[:, :])
            loop_ctx = tc.For_i(0, repeat, 1) if repeat else contextlib.nullcontext()
            with loop_ctx:
              for _ in range(unroll if repeat else 1):
                for t in TILE_ORDER:
                    last = t == TILE_ORDER[-1]
                    rt = relp.tile([128, W], f32, tag="rt")
                    if last:
                        nc.sync.dma_start(out=rt[:, :LAST_SPLIT],
                                          in_=rel[t * 128:(t + 1) * 128, :LAST_SPLIT])
                    else:
                        nc.sync.dma_start(out=rt[:, :],
                                          in_=rel[t * 128:(t + 1) * 128, :])
                    # ScalarE: "any" over the suffix [512t, 4096); columns
                    # below 512t are covered by the prefix max via
                    # full_any = earlier_any | suffix_any.
                    if last:
                        nc.scalar.activation(
                            out=dummy.broadcast_to((128, LAST_SPLIT - t * CW)),
                            in_=rt[:, t * CW:LAST_SPLIT],
                            func=ACT.Relu, bias=biasm[:, :], scale=1.0,
                            accum_out=out_t[:, t:t + 1])
                    else:
                        nc.scalar.activation(
                            out=dummy.broadcast_to((128, W - t * CW)),
                            in_=rt[:, t * CW:],
                            func=ACT.Relu, bias=biasm[:, :], scale=1.0,
                            accum_out=out_t[:, t:t + 1])
                    # Prefix "any" over the fully-earlier columns [0, 512t).
                    if t in SCALAR_PREFIX:
                        nc.scalar.activation(
                            out=dummy.broadcast_to((128, t * CW)),
                            in_=rt[:, 0:t * CW],
                            func=ACT.Relu, bias=biasm[:, :], scale=1.0,
                            accum_out=out_t[:, NT + t:NT + t + 1])
                    elif t > 0:
                        nc.vector.tensor_reduce(
                            out=out_t[:, NT + t:NT + t + 1],
                            in_=rt[:, 0:t * CW], axis=AX.X, op=OP.max)
                    # VectorE: masked max over the diagonal chunk.
                    sc = scr.tile([128, CW], f32)
                    nc.vector.tensor_tensor(
                        out=sc[:, :], in0=rt[:, t * CW:(t + 1) * CW],
                        in1=mask[:, :], op=OP.mult)
                    nc.vector.tensor_reduce(
                        out=out_t[:, 2 * NT + t:2 * NT + t + 1],
                        in_=sc[:, :], axis=AX.X, op=OP.max)
                    if last:
                        # Piece B of the split last tile: the remaining
                        # suffix columns, streamed + reduced after piece A.
                        nc.sync.dma_start(out=rt[:, LAST_SPLIT:],
                                          in_=rel[t * 128:(t + 1) * 128, LAST_SPLIT:])
                        nc.scalar.activation(
                            out=dummy.broadcast_to((128, W - LAST_SPLIT)),
                            in_=rt[:, LAST_SPLIT:],
                            func=ACT.Relu, bias=biasm[:, :], scale=1.0,
                            accum_out=out_t[:, 3 * NT:3 * NT + 1])
            nc.sync.dma_start(out=outmax[:, :], in_=out_t[:, :])
    nc.compile()
    return nc


def _get_nc():
    if "nc" not in _CACHE:
        _CACHE["nc"] = _build_nc()
    return _CACHE["nc"]


def _trimask():
    if "mask" not in _CACHE:
        li = np.arange(128)[:, None]          # local row index
        f = np.arange(CW)[None, :]            # free index = local_p*4 + k
        _CACHE["mask"] = (f < 4 * li).astype(np.float32)
    return _CACHE["mask"]


def unpack_outmax(om):
    """om: [128, 3*NT] -> (full_any [S], earlier_any [S]) bools; row i = t*128+p.

    ScalarE sums only cover the suffix columns [512t, 4096); the prefix is
    covered by the GpSimd earlier-max, and the masked diagonal lies inside
    the suffix, so  full_any = earlier_any | suffix_any  exactly."""
    sums = om[:, :NT].copy()
    sums[:, TILE_ORDER[-1]] += om[:, 3 * NT]   # second piece of split tile
    suffix = (sums > 0.0).T.reshape(S)                       # [t, p] -> i
    pthr = np.where(np.isin(np.arange(NT), SCALAR_PREFIX), 0.0, 0.5)
    earlier = ((om[:, NT:2 * NT] > pthr[None, :]) |
               (om[:, 2 * NT:2 * NT + NT] > 0.5)).T.reshape(S)
    return suffix | earlier, earlier


def run_device(rel, trace=False):
    """rel: [B, S, S, 4] f32.  Returns (full_any, earlier_any [B,S] bool, results)."""
    from concourse.bass_utils import run_bass_kernel_spmd

    nc = _get_nc()
    mask = _trimask()
    in_maps = [
        {"rel": np.ascontiguousarray(rel[b].reshape(S, W)), "trimask": mask}
        for b in range(B)
    ]
    res = run_bass_kernel_spmd(nc, in_maps, core_ids=list(range(B)), trace=trace)
    full = np.empty((B, S), dtype=bool)
    earlier = np.empty((B, S), dtype=bool)
    for b, r in enumerate(res.results):
        full[b], earlier[b] = unpack_outmax(r["outmax"])
    return full, earlier, res


# --------------------------------------------------------------------------
# host epilogue: O(B*S) factor logic + closed-form scan
# --------------------------------------------------------------------------

def _host_forward(aL, oL, full_any, earlier_any, exA, exO,
                  W1, b1, W2, b2, W3, b3):
    B_, S_, _ = aL.shape
    x = np.concatenate([aL, oL], axis=-1)
    h = np.maximum(x @ W1 + b1, 0.0).astype(np.float32)
    h = np.maximum(h @ W2 + b2, 0.0).astype(np.float32)
    z = (h @ W3 + b3)[..., 0].astype(np.float32)
    c = (1.0 / (1.0 + np.exp(-z.astype(np.float64)))).astype(np.float32)
    mult1 = np.where(c < 0.5, np.float32(2.0) * c, np.float32(1.0)).astype(np.float32)

    def window_any(flag, w):
        out = np.zeros_like(flag)
        for d in range(-w, w + 1):
            if d < 0:
                out[:, :d] |= flag[:, -d:]
            elif d > 0:
                out[:, d:] |= flag[:, :-d]
            else:
                out |= flag
        return out

    nearA = window_any(exA > 0, 3)
    nearO = window_any(exO > 0, 3)
    e = np.exp((aL - aL.max(-1, keepdims=True)).astype(np.float32))
    impA = e[..., :2].sum(-1) / e.sum(-1)
    e = np.exp((oL - oL.max(-1, keepdims=True)).astype(np.float32))
    impO = e[..., :2].sum(-1) / e.sum(-1)
    factA2 = np.where((impA > 0.5) & ~nearO, np.float32(0.3), np.float32(1.0))
    factO2 = np.where((impO > 0.5) & ~nearA, np.float32(0.3), np.float32(1.0))

    factA4 = np.where(full_any & earlier_any, np.float32(0.7), np.float32(1.0))

    # ---- scan closed form ----
    actO = (aL.max(-1) > 0.5) | (oL.max(-1) > 0.5)
    n = np.zeros((B_, S_), dtype=bool)
    n[:, :-1] |= actO[:, 1:]
    n[:, :-2] |= actO[:, 2:]

    def act_of(fa, fo):
        aRow = aL * mult1[..., None]
        oRow = oL * mult1[..., None]
        aRow[..., :2] *= fa[..., None]
        oRow[..., :2] *= fo[..., None]
        return (aRow.max(-1) > 0.5) | (oRow.max(-1) > 0.5)

    u1 = act_of((factA2 * np.float32(1.0)) * factA4, factO2 * np.float32(1.0))
    u0 = act_of((factA2 * np.float32(0.1)) * factA4, factO2 * np.float32(0.1))

    k = ~u1
    src = u0 | (u1 & n)
    DK = np.zeros((B_, S_), dtype=bool)
    DK[:, 1:] = k[:, 1:] & k[:, :-1]

    idx = np.arange(S_)[None, :]
    LS = np.maximum.accumulate(np.where(src, idx, -1), axis=1)
    LDK = np.maximum.accumulate(np.where(DK, idx, -1), axis=1)
    a = u1 & (LS >= 0) & (LS >= LDK)

    r = n.copy()
    r[:, 1:] |= a[:, :-1]
    r[:, 2:] |= a[:, :-2]
    fact3 = np.where(~r, np.float32(0.1), np.float32(1.0))

    fa = (factA2 * fact3) * factA4
    fo = factO2 * fact3
    cA = aL * mult1[..., None]
    cO = oL * mult1[..., None]
    cA[..., :2] *= fa[..., None]
    cO[..., :2] *= fo[..., None]
    return cA.astype(np.float32), cO.astype(np.float32)


# --------------------------------------------------------------------------
# entry point
# --------------------------------------------------------------------------

def kernel(aspect_logits, opinion_logits, aspect_opinion_relations,
           explicit_aspects, explicit_opinions, W1, b1, W2, b2, W3, b3):
    aL = np.asarray(aspect_logits, dtype=np.float32)
    oL = np.asarray(opinion_logits, dtype=np.float32)
    rel = np.asarray(aspect_opinion_relations, dtype=np.float32)
    exA = np.asarray(explicit_aspects)
    exO = np.asarray(explicit_opinions)
    full_any, earlier_any, _ = run_device(rel)
    return _host_forward(
        aL, oL, full_any, earlier_any, exA, exO,
        np.asarray(W1, np.float32), np.asarray(b1, np.float32),
        np.asarray(W2, np.float32), np.asarray(b2, np.float32),
        np.asarray(W3, np.float32), np.asarray(b3, np.float32))
